# revision 6
# baseline (speedup 1.0000x reference)
"""Cumulative link (ordinal) loss on 8 Trainium2 NeuronCores.

loss = mean_i [ -ln( sigmoid(hi_i - x_i) - sigmoid(lo_i - x_i) + eps ) ]
with per-label thresholds hi = [0,1,2,3,+inf][l], lo = [-inf,0,1,2,3][l].

Device formulation (lm = l - 3 bf16, H = lm - x computed f32 on PE):
    H  = I @ lm - I @ x      # TensorE identity matmuls, PSUM f32
    S1 = sigmoid(H + 3)      # = sigmoid(hi - x) when l <= 3   (ACT bias)
    S2 = sigmoid(H + 2)      # = sigmoid(lo - x) when l >= 1   (ACT bias)
    A  = max(lm, S1)         # l==4  ->  1,  else S1
    B  = min(lev, S2)        # l==0  ->  0,  else S2   (lev = lm + 3)
    P  = A - B
    sum ln over 8-way products of P (one Ln pass; the 1e-38 bias only
    rescues exact-zero 16-bit sigmoid-saturation collisions, ~3e-4 rel;
    a larger bias would clamp legitimate p^8 values, which reach 6e-21).

Engine balance per core (1M elements), all under the 29.3us DMA floor:
  * DMA: 12.6 MB HBM reads on the single sync HWDGE ring, measured at
    ~430 GB/s sustained.  No SWDGE (Q7 descgen contends with DVE
    perf-mode SBUF ports; int64-cast DMA hard-crashes the core).
    Ragged chunks: small first chunk starts compute earlier, small last
    chunks shrink the post-DMA tail.
  * TensorE (~18us, otherwise idle) eats the raw f32 logits via
    (-I) @ x + I @ lm accumulated in PSUM -- this removes both the
    f32->16-bit cast (8.8us of ACT Copy in the previous rev) and the
    DVE H-subtract, and the sigmoids read PSUM at full ACT rate.
  * DVE (~26us): lm (int32->bf16, 2x), lev (4x), A, B, P and the
    product tree, every tensor_tensor dense bf16 so the 2x perf mode
    holds (strided/f32/PSUM operands all drop to 1x -- measured).
  * ACT (~20us): 2 sigmoid passes + one Ln + exactly two table loads,
    the Ln load hidden under the DVE tree tail.

Sharding: pure data parallel, 1/8 of batch per core, [128 x 8192].
"""

import numpy as np

B_TOTAL = 8388608
N_CORES = 8
P = 128
SHARD = B_TOTAL // N_CORES          # 1048576 per core
M = SHARD // P                      # 8192 free-dim columns per core
CHUNKS = [512, 1536, 2048, 2048, 1024, 512, 512]   # sum = M
EPS_LN = 1e-38

_NC = None


def _build_nc():
    import concourse.bacc as bacc
    import concourse.mybir as mybir
    from concourse import tile
    from concourse.tile_rust import add_dep_helper

    f32 = mybir.dt.float32
    bf16 = mybir.dt.bfloat16
    i32 = mybir.dt.int32
    Alu = mybir.AluOpType
    Act = mybir.ActivationFunctionType

    nc = bacc.Bacc("TRN2", target_bir_lowering=False, debug=False,
                   enable_asserts=False)

    x_dram = nc.dram_tensor("logits", (P, M), f32, kind="ExternalInput")
    # int32 pairs at the PJRT boundary (int64 inputs crash the axon run
    # path); low word of each pair is the label value.
    l_dram = nc.dram_tensor("labels", (P, 2 * M), i32, kind="ExternalInput")
    o_dram = nc.dram_tensor("out", (P, 1), f32, kind="ExternalOutput")

    NCH = len(CHUNKS)
    starts = np.cumsum([0] + CHUNKS).tolist()

    with tile.TileContext(nc) as tc:
        with tc.tile_pool(name="io", bufs=3) as iop, \
             tc.tile_pool(name="work", bufs=3) as wp, \
             tc.psum_pool(name="ps", bufs=2) as psp, \
             tc.tile_pool(name="persist", bufs=1) as pp:
            bias3 = pp.tile([P, 1], f32, tag="bias3")
            nc.vector.memset(bias3[:], 3.0)
            bias2 = pp.tile([P, 1], f32, tag="bias2")
            nc.vector.memset(bias2[:], 2.0)
            bias_eps = pp.tile([P, 1], f32, tag="bias_eps")
            nc.vector.memset(bias_eps[:], EPS_LN)
            ineg = pp.tile([P, P], f32, tag="ineg")       # -I for x
            nc.vector.memset(ineg[:], 0.0)
            nc.gpsimd.affine_select(out=ineg[:], in_=ineg[:],
                                    compare_op=Alu.not_equal, fill=-1.0,
                                    base=0, pattern=[[-1, P]],
                                    channel_multiplier=1)
            idb = pp.tile([P, P], bf16, tag="idb")        # +I for lm
            nc.vector.memset(idb[:], 0.0)
            nc.gpsimd.affine_select(out=idb[:], in_=idb[:],
                                    compare_op=Alu.not_equal, fill=1.0,
                                    base=0, pattern=[[-1, P]],
                                    channel_multiplier=1)
            pf = pp.tile([P, M], bf16, tag="pf")          # P values
            q1 = pp.tile([P, M // 2], bf16, tag="q1")     # pair products
            q2 = pp.tile([P, M // 4], bf16, tag="q2")     # 4-way
            q3 = pp.tile([P, M // 8], bf16, tag="q3")     # 8-way
            acc = pp.tile([P, 1], f32, tag="acc")

            # --- DMA: all chunks interleaved on the sync HWDGE ring ---
            l32s, xts = [], []
            for c in range(NCH):
                w = CHUNKS[c]
                s = starts[c]
                l32 = iop.tile([P, 2048, 2], i32, tag="l32")
                xt = iop.tile([P, 2048], f32, tag="xt")
                nc.sync.dma_start(out=l32[:, :w, :],
                                  in_=l_dram[:, 2 * s:2 * (s + w)])
                nc.sync.dma_start(out=xt[:, :w], in_=x_dram[:, s:s + w])
                l32s.append(l32)
                xts.append(xt)

            acts = []            # ACT program order (pinned below)

            def emit_tile(c):
                w = CHUNKS[c]
                s = starts[c]
                ls = l32s[c][:, :w, 0]
                lm = wp.tile([P, 2048], bf16, tag="lm")
                lev = wp.tile([P, 2048], bf16, tag="lev")
                s1 = wp.tile([P, 2048], bf16, tag="s1")
                s2 = wp.tile([P, 2048], bf16, tag="s2")
                h = psp.tile([P, 2048], f32, tag="h")
                # lm = l - 3    (int32 strided -> dense bf16, 2x)
                nc.vector.tensor_scalar(out=lm[:, :w], in0=ls, scalar1=-3.0,
                                        scalar2=None, op0=Alu.add)
                # lev = l      (dense 4x)
                nc.vector.tensor_scalar(out=lev[:, :w], in0=lm[:, :w],
                                        scalar1=3.0, scalar2=None, op0=Alu.add)
                # H = lm - x   (TensorE, PSUM f32; 512-col blocks = 1 bank)
                for k in range(0, w, 512):
                    nc.tensor.matmul(h[:, k:k + 512], ineg[:],
                                     xts[c][:, k:k + 512],
                                     start=True, stop=False)
                    nc.tensor.matmul(h[:, k:k + 512], idb[:],
                                     lm[:, k:k + 512],
                                     start=False, stop=True)
                acts.append(nc.scalar.activation(s1[:, :w], h[:, :w],
                                                 Act.Sigmoid, bias=bias3[:]))
                acts.append(nc.scalar.activation(s2[:, :w], h[:, :w],
                                                 Act.Sigmoid, bias=bias2[:]))
                # A = max(lm, S1) -> s1 slot
                nc.vector.tensor_tensor(out=s1[:, :w], in0=lm[:, :w],
                                        in1=s1[:, :w], op=Alu.max)
                # B = min(lev, S2) -> s2 slot
                nc.vector.tensor_tensor(out=s2[:, :w], in0=lev[:, :w],
                                        in1=s2[:, :w], op=Alu.min)
                # P -> pf[:, s:s+w]
                nc.vector.tensor_tensor(out=pf[:, s:s + w], in0=s1[:, :w],
                                        in1=s2[:, :w], op=Alu.subtract)

            def fold(dst, src, lo, width):
                hw = width // 2
                nc.vector.tensor_tensor(
                    out=dst[:, lo // 2:lo // 2 + hw],
                    in0=src[:, lo:lo + hw], in1=src[:, lo + hw:lo + width],
                    op=Alu.mult)

            # Emission order = availability order.  Chunk cumulative ends:
            # 512, 2048, 4096, 6144, 7168, 7680, 8192.
            emit_tile(0)
            emit_tile(1)                      # pf[0:2048] ready
            fold(q1, pf, 0, 2048)             # q1[0:1024]
            emit_tile(2)                      # pf[0:4096]
            fold(q1, pf, 2048, 2048)          # q1[1024:2048]
            fold(q2, q1, 0, 2048)             # q2[0:1024]
            emit_tile(3)                      # pf[0:6144]
            fold(q1, pf, 4096, 2048)          # q1[2048:3072]
            emit_tile(4)                      # pf[0:7168]
            fold(q1, pf, 6144, 1024)          # q1[3072:3584]
            fold(q2, q1, 2048, 1024)          # q2[1024:1536]
            emit_tile(5)                      # pf[0:7680]
            emit_tile(6)                      # pf full
            fold(q1, pf, 7168, 1024)          # q1[3584:4096]
            fold(q2, q1, 3072, 1024)          # q2[1536:2048]
            fold(q3, q2, 0, 2048)             # q3 = 8-way products
            acts.append(nc.scalar.activation(q3[:], q3[:], Act.Ln,
                                             bias=bias_eps[:],
                                             accum_out=acc[:]))

            # Freeze ACT program order exactly as emitted: sigmoid table
            # loads once up front, ln loads once at the end (hidden under
            # the DVE product-tree tail).
            for prev, nxt in zip(acts, acts[1:]):
                add_dep_helper(nxt.ins, prev.ins, sync=False,
                               reason="pin ACT order")

            nc.sync.dma_start(out=o_dram[:], in_=acc[:])

    nc.compile()
    return nc


def get_nc():
    global _NC
    if _NC is None:
        _NC = _build_nc()
    return _NC


def make_in_maps(logits, labels):
    x = np.ascontiguousarray(np.asarray(logits, dtype=np.float32)).reshape(B_TOTAL)
    lab = np.asarray(labels)
    if lab.dtype != np.int64:
        lab = lab.astype(np.int64)
    lab = np.ascontiguousarray(lab).reshape(B_TOTAL)
    in_maps = []
    for c in range(N_CORES):
        xs = x[c * SHARD:(c + 1) * SHARD].reshape(P, M)
        ls = lab[c * SHARD:(c + 1) * SHARD].view(np.int32).reshape(P, 2 * M)
        in_maps.append({"logits": xs, "labels": ls})
    return in_maps


def run(logits, labels, trace=False):
    """Returns (loss_scalar_f32, BassKernelResults)."""
    from concourse.bass_utils import run_bass_kernel_spmd

    nc = get_nc()
    in_maps = make_in_maps(logits, labels)
    res = run_bass_kernel_spmd(
        nc, in_maps, core_ids=list(range(N_CORES)), trace=trace
    )
    total = 0.0
    for r in res.results:
        total += r["out"].astype(np.float64).sum()
    loss = np.float32(-total / B_TOTAL)
    return np.asarray(loss), res


def kernel(logits, labels):
    out, _ = run(logits, labels, trace=False)
    return out
